# revision 23
# baseline (speedup 1.0000x reference)
"""AFNO1D Trainium2 kernel: FFT->block-MLP->softshrink->IFFT->residual.

Strategy: the FFT along C is linear, so it is fused into the layer-1
weights on the host (W1_eff = DFT_block @ w1); the IFFT's real output is
built from half-spectrum matmuls A = o2r@cos, B = o2i@sin with
out[c] = A-B+x and out[1024-c] = A+B+x[1024-c] (reversal done by host
indexing). Everything on-chip is matmul + elementwise, computed
channel-major so the contraction dim sits on SBUF partitions.

Data-parallel over B=8: core b handles x[b]; params replicated; no
collectives. Host transposes shards in/out.

Compute dtype: fp8(e4m3) operands in DoubleRow mode for the two dense
layers, bf16 for the small block-diagonal layer, fp32 PSUM + fp32
residual path. All quantization scales fold into host-prepared weights
and biases; the graph itself has only two fixed descale constants.
"""

from contextlib import ExitStack

import numpy as np
import ml_dtypes

import concourse.bass as bass
import concourse.mybir as mybir
import concourse.tile as tile
from concourse import bacc
from concourse.bass_utils import run_bass_kernel_spmd

HIDDEN = 1024
NB = 8          # channel blocks
BS = 128        # block size
LAM = 0.01
N_CORES = 8
NROWS = 4096    # rows (sequence positions) per core
R = 512         # rows per chunk
NCHUNK = NROWS // R

FP8 = True      # fp8 DoubleRow for layers 1/3 (bf16 fallback if False)
SWI = False     # DoubleRowSwInterleave: 21% faster LDW in micro, but both
                # interp-style and natural weight layouts fail numerics on HW
SX = 8.0        # x -> fp8 scale (|x| up to 30 before clipping)
SW1 = 512.0     # W1_eff -> fp8 scale (|W1| up to 0.47)
SO = 32.0       # o1 and o2 -> fp8 scale factor (via SW2 = SO)
SG = 2048.0     # IFFT cos/sin -> fp8 scale (entries <= 1/32 -> <= 64)
ALU = mybir.AluOpType

F32 = mybir.dt.float32
BF16 = mybir.dt.bfloat16
E4 = mybir.dt.float8e4
DR = (mybir.MatmulPerfMode.DoubleRowSwInterleave if SWI
      else mybir.MatmulPerfMode.DoubleRow)
RELU = mybir.ActivationFunctionType.Relu

_GRAPH_CACHE = {}


def _build_graph(rep=1, fp8=FP8, mmx=1):
    key = ("nc", rep, fp8, mmx)
    if key in _GRAPH_CACHE:
        return _GRAPH_CACHE[key]

    WDT = E4 if fp8 else BF16

    nc = bacc.Bacc("TRN2", target_bir_lowering=False, debug=False,
                   num_devices=N_CORES)

    xt = nc.dram_tensor("xt", [NB, BS, NROWS], F32, kind="ExternalInput").ap()
    xt8 = nc.dram_tensor("xt8", [NB, BS, NROWS], E4 if fp8 else BF16,
                         kind="ExternalInput").ap()
    w1r = nc.dram_tensor("w1r", [NB, BS, HIDDEN], WDT, kind="ExternalInput").ap()
    w1i = nc.dram_tensor("w1i", [NB, BS, HIDDEN], WDT, kind="ExternalInput").ap()
    gcp = nc.dram_tensor("gcp", [NB // 2, BS, 2, HIDDEN // 2], WDT, kind="ExternalInput").ap()
    gsp = nc.dram_tensor("gsp", [NB // 2, BS, 2, HIDDEN // 2], WDT, kind="ExternalInput").ap()
    g512 = nc.dram_tensor("g512", [BS, NB], WDT, kind="ExternalInput").ap()
    xtr = nc.dram_tensor("xtr", [NB // 2, BS, NROWS], F32, kind="ExternalInput").ap()
    w2rp = nc.dram_tensor("w2rp", [NB, BS, 2, BS], WDT, kind="ExternalInput").ap()
    w2ip = nc.dram_tensor("w2ip", [NB, BS, 2, BS], WDT, kind="ExternalInput").ap()
    b1r = nc.dram_tensor("b1r", [BS, NB], F32, kind="ExternalInput").ap()
    b1i = nc.dram_tensor("b1i", [BS, NB], F32, kind="ExternalInput").ap()
    b2m1r = nc.dram_tensor("b2m1r", [BS, NB], F32, kind="ExternalInput").ap()
    b2m1i = nc.dram_tensor("b2m1i", [BS, NB], F32, kind="ExternalInput").ap()
    b2m2r = nc.dram_tensor("b2m2r", [BS, NB], F32, kind="ExternalInput").ap()
    b2m2i = nc.dram_tensor("b2m2i", [BS, NB], F32, kind="ExternalInput").ap()
    outl = nc.dram_tensor("outl", [NB // 2, BS, NROWS], F32, kind="ExternalOutput").ap()
    outh = nc.dram_tensor("outh", [NB // 2, BS, NROWS], F32, kind="ExternalOutput").ap()
    out5 = nc.dram_tensor("out5", [1, NROWS], F32, kind="ExternalOutput").ap()

    # graph descale constants (everything else is folded host-side)
    K3 = 1.0 / (SO * SG) if fp8 else 1.0
    K5 = 1.0 / (SO * 32.0) if fp8 else 1.0
    SIG = 1.0 / (SX * SW1) if fp8 else 1.0   # L1 psum descale (o1 at true scale)

    with tile.TileContext(nc) as tc, ExitStack() as ctx:
        wpool = ctx.enter_context(tc.tile_pool(name="weights", bufs=1))
        w1r_sb = wpool.tile([BS, NB, HIDDEN], WDT, tag="w1r", name="w1r_sb")
        w1i_sb = wpool.tile([BS, NB, HIDDEN], WDT, tag="w1i", name="w1i_sb")
        gcp_sb = wpool.tile([BS, NB // 2, 2, HIDDEN // 2], WDT, tag="gcp", name="gcp_sb")
        gsp_sb = wpool.tile([BS, NB // 2, 2, HIDDEN // 2], WDT, tag="gsp", name="gsp_sb")
        g512_sb = wpool.tile([BS, NB], WDT, tag="g512", name="g512_sb")
        for ci in range(NB):
            nc.sync.dma_start(out=w1r_sb[:, ci, :], in_=w1r[ci])
            nc.sync.dma_start(out=w1i_sb[:, ci, :], in_=w1i[ci])
        w2rp_sb = wpool.tile([BS, NB, 2, BS], WDT, tag="w2rp", name="w2rp_sb")
        w2ip_sb = wpool.tile([BS, NB, 2, BS], WDT, tag="w2ip", name="w2ip_sb")

        def _late_weight_dmas():
            nc.sync.dma_start(out=g512_sb[:], in_=g512[:])
            for kb in range(NB):
                nc.sync.dma_start(out=w2rp_sb[:, kb, :, :], in_=w2rp[kb])
                nc.sync.dma_start(out=w2ip_sb[:, kb, :, :], in_=w2ip[kb])
            for t in range(NB // 2):
                nc.sync.dma_start(out=gcp_sb[:, t, :, :], in_=gcp[t])
                nc.sync.dma_start(out=gsp_sb[:, t, :, :], in_=gsp[t])
        bias_tiles = {}
        for nm, ap in (("b1r", b1r), ("b1i", b1i), ("b2m1r", b2m1r),
                       ("b2m1i", b2m1i), ("b2m2r", b2m2r), ("b2m2i", b2m2i)):
            t = wpool.tile([BS, NB], F32, tag=nm, name=f"{nm}_sb")
            nc.sync.dma_start(out=t[:], in_=ap[:])
            bias_tiles[nm] = t

        xpool = ctx.enter_context(tc.tile_pool(name="xin", bufs=2))
        xqpool = ctx.enter_context(tc.tile_pool(name="xq", bufs=3))
        bfpool = ctx.enter_context(tc.tile_pool(name="bf", bufs=2))
        opool = ctx.enter_context(tc.tile_pool(name="acts", bufs=2))
        outpool = ctx.enter_context(tc.tile_pool(name="outs", bufs=2))
        ppool = ctx.enter_context(tc.tile_pool(name="psum", bufs=4, space="PSUM"))

        XDT = E4 if fp8 else BF16
        for ch in range(NCHUNK * rep):
            r0 = (ch % NCHUNK) * R
            xt_q = xqpool.tile([BS, NB, R], XDT, tag="xt_q", name=f"xt_q{ch}")
            for ci in range(NB):
                nc.sync.dma_start(out=xt_q[:, ci, :], in_=xt8[ci, :, r0:r0 + R])
            xt_f = xpool.tile([BS, NB, R], F32, tag="xt_f", name=f"xt_f{ch}")
            for ci in range(NB):
                nc.sync.dma_start(out=xt_f[:, ci, :], in_=xt[ci, :, r0:r0 + R])
            xr_f = xpool.tile([BS, NB // 2, R], F32, tag="xr_f", name=f"xr_f{ch}")
            for ci in range(NB // 2):
                nc.sync.dma_start(out=xr_f[:, ci, :], in_=xtr[ci, :, r0:r0 + R])
            if ch == 0:
                _late_weight_dmas()

            # layer 1 (FFT fused): o1 = relu(SIG*psum + b1), stored paired fp8
            o1p = opool.tile([BS, NB, 2, R], XDT, tag="o1p", name=f"o1p{ch}")
            for ro in range(NB):
                pr = ppool.tile([BS, R], F32, tag="pr", name=f"pr{ch}_{ro}")
                pi = ppool.tile([BS, R], F32, tag="pi", name=f"pi{ch}_{ro}")
                cs = slice(ro * BS, (ro + 1) * BS)
                if fp8:
                    for t in range(NB // 2):
                        for j in range(mmx):
                            nc.tensor.matmul(pr[:], w1r_sb[:, 2 * t:2 * t + 2, cs],
                                             xt_q[:, 2 * t:2 * t + 2, :],
                                             start=(t == 0 and j == 0),
                                             stop=(t == NB // 2 - 1 and j == mmx - 1),
                                             perf_mode=DR)
                    for t in range(NB // 2):
                        for j in range(mmx):
                            nc.tensor.matmul(pi[:], w1i_sb[:, 2 * t:2 * t + 2, cs],
                                             xt_q[:, 2 * t:2 * t + 2, :],
                                             start=(t == 0 and j == 0),
                                             stop=(t == NB // 2 - 1 and j == mmx - 1),
                                             perf_mode=DR)
                else:
                    for ci in range(NB):
                        nc.tensor.matmul(pr[:], w1r_sb[:, ci, cs], xt_q[:, ci, :],
                                         start=(ci == 0), stop=(ci == NB - 1))
                    for ci in range(NB):
                        nc.tensor.matmul(pi[:], w1i_sb[:, ci, cs], xt_q[:, ci, :],
                                         start=(ci == 0), stop=(ci == NB - 1))
                nc.scalar.activation(o1p[:, ro, 0, :], pr[:], RELU, scale=SIG,
                                     bias=bias_tiles["b1r"][:, ro:ro + 1])
                nc.scalar.activation(o1p[:, ro, 1, :], pi[:], RELU, scale=SIG,
                                     bias=bias_tiles["b1i"][:, ro:ro + 1])

            # layer 2 (block-diag complex, fp8 DR on paired o1) + softshrink:
            # softshrink(z) = relu(z + b2 - lam) - relu(-z - b2 - lam), z = SO*o2pre
            o2r = opool.tile([BS, NB, R], XDT, tag="o2r", name=f"o2r{ch}")
            o2i = opool.tile([BS, NB, R], XDT, tag="o2i", name=f"o2i{ch}")
            for kb in range(NB):
                qr = ppool.tile([BS, R], F32, tag="pr", name=f"qr{ch}_{kb}")
                qi = ppool.tile([BS, R], F32, tag="pi", name=f"qi{ch}_{kb}")
                if fp8:
                    for j in range(mmx):
                        nc.tensor.matmul(qr[:], w2rp_sb[:, kb, :, :], o1p[:, kb, :, :],
                                         start=(j == 0), stop=(j == mmx - 1),
                                         perf_mode=DR)
                    for j in range(mmx):
                        nc.tensor.matmul(qi[:], w2ip_sb[:, kb, :, :], o1p[:, kb, :, :],
                                         start=(j == 0), stop=(j == mmx - 1),
                                         perf_mode=DR)
                else:
                    nc.tensor.matmul(qr[:], w2rp_sb[:, kb, 0, :], o1p[:, kb, 0, :],
                                     start=True, stop=False)
                    nc.tensor.matmul(qr[:], w2rp_sb[:, kb, 1, :], o1p[:, kb, 1, :],
                                     start=False, stop=True)
                    nc.tensor.matmul(qi[:], w2ip_sb[:, kb, 0, :], o1p[:, kb, 0, :],
                                     start=True, stop=False)
                    nc.tensor.matmul(qi[:], w2ip_sb[:, kb, 1, :], o1p[:, kb, 1, :],
                                     start=False, stop=True)
                t1r = bfpool.tile([BS, R], BF16, tag="t1r", name=f"t1r{ch}_{kb}")
                t1i = bfpool.tile([BS, R], BF16, tag="t1i", name=f"t1i{ch}_{kb}")
                t2r = bfpool.tile([BS, R], BF16, tag="t2r", name=f"t2r{ch}_{kb}")
                t2i = bfpool.tile([BS, R], BF16, tag="t2i", name=f"t2i{ch}_{kb}")
                nc.vector.tensor_scalar(t1r[:], qr[:],
                                        bias_tiles["b2m1r"][:, kb:kb + 1], 0.0,
                                        ALU.add, ALU.max)
                nc.vector.tensor_scalar(t1i[:], qi[:],
                                        bias_tiles["b2m1i"][:, kb:kb + 1], 0.0,
                                        ALU.add, ALU.max)
                nc.scalar.activation(t2r[:], qr[:], RELU, scale=-1.0,
                                     bias=bias_tiles["b2m2r"][:, kb:kb + 1])
                nc.scalar.activation(t2i[:], qi[:], RELU, scale=-1.0,
                                     bias=bias_tiles["b2m2i"][:, kb:kb + 1])
                nc.vector.tensor_sub(o2r[:, kb, :], t1r[:], t2r[:])
                nc.vector.tensor_sub(o2i[:, kb, :], t1i[:], t2i[:])

            # layer 3 (IFFT real part, half-spectrum): pA = SO*SG*A, pB = -SO*SG*B
            # outl[c] = (K3*pA + x) + K3*pB;  outh[c]=out[1024-c] = (K3*pA + xr) - K3*pB
            outl_f = outpool.tile([BS, NB // 2, R], F32, tag="outl_f", name=f"outl_f{ch}")
            outh_f = outpool.tile([BS, NB // 2, R], F32, tag="outh_f", name=f"outh_f{ch}")
            for co in range(NB // 2):
                pA = ppool.tile([BS, R], F32, tag="pr", name=f"pA{ch}_{co}")
                pB = ppool.tile([BS, R], F32, tag="pi", name=f"pB{ch}_{co}")
                cs = slice(co * BS, (co + 1) * BS)
                if fp8:
                    for t in range(NB // 2):
                        nc.tensor.matmul(pA[:], gcp_sb[:, t, :, cs],
                                         o2r[:, 2 * t:2 * t + 2, :],
                                         start=(t == 0), stop=(t == NB // 2 - 1),
                                         perf_mode=DR)
                    for t in range(NB // 2):
                        nc.tensor.matmul(pB[:], gsp_sb[:, t, :, cs],
                                         o2i[:, 2 * t:2 * t + 2, :],
                                         start=(t == 0), stop=(t == NB // 2 - 1),
                                         perf_mode=DR)
                else:
                    for t in range(NB // 2):
                        for j in range(2):
                            nc.tensor.matmul(pA[:], gcp_sb[:, t, j, cs],
                                             o2r[:, 2 * t + j, :],
                                             start=(t == 0 and j == 0),
                                             stop=(t == NB // 2 - 1 and j == 1))
                    for t in range(NB // 2):
                        for j in range(2):
                            nc.tensor.matmul(pB[:], gsp_sb[:, t, j, cs],
                                             o2i[:, 2 * t + j, :],
                                             start=(t == 0 and j == 0),
                                             stop=(t == NB // 2 - 1 and j == 1))
                tl_ = bfpool.tile([BS, R], F32, tag="tmp3l", name=f"tl_{ch}_{co}")
                th_ = bfpool.tile([BS, R], F32, tag="tmp3h", name=f"th_{ch}_{co}")
                nc.vector.scalar_tensor_tensor(tl_[:], pA[:], K3, xt_f[:, co, :],
                                               ALU.mult, ALU.add)
                nc.vector.scalar_tensor_tensor(th_[:], pA[:], K3, xr_f[:, co, :],
                                               ALU.mult, ALU.add)
                nc.vector.scalar_tensor_tensor(outl_f[:, co, :], pB[:], K3, tl_[:],
                                               ALU.mult, ALU.add)
                nc.vector.scalar_tensor_tensor(outh_f[:, co, :], pB[:], -K3, th_[:],
                                               ALU.mult, ALU.add)
                nc.sync.dma_start(out=outl[co, :, r0:r0 + R], in_=outl_f[:, co, :])
                nc.sync.dma_start(out=outh[co, :, r0:r0 + R], in_=outh_f[:, co, :])
            # channel 512: out[512] = K5 * sum_k (+-1)(SO*o2r[k]) + x[512]
            p5 = ppool.tile([1, R], F32, tag="pi", name=f"p5_{ch}")
            for kb in range(NB):
                nc.tensor.matmul(p5[:], g512_sb[:, kb:kb + 1], o2r[:, kb, :],
                                 start=(kb == 0), stop=(kb == NB - 1))
            o5 = outpool.tile([1, R], F32, tag="out5_f", name=f"o5_{ch}")
            nc.vector.scalar_tensor_tensor(o5[:], p5[:], K5, xt_f[0:1, 4, :],
                                           ALU.mult, ALU.add)
            nc.sync.dma_start(out=out5[0:1, r0:r0 + R], in_=o5[:])

    nc.compile()
    _GRAPH_CACHE[key] = nc
    return nc


def _build_host_weights(w1, b1, w2, b2, fp8=FP8):
    C = HIDDEN
    k = np.arange(C)
    c = np.arange(C)
    ph = (np.outer(c, k) % C).astype(np.float64) * (2.0 * np.pi / C)
    s = 1.0 / np.sqrt(C)
    Fr = np.cos(ph) * s        # [c, k]
    Fi = -np.sin(ph) * s
    w1 = np.asarray(w1, np.float64)
    W1r = np.empty((C, C), np.float64)
    W1i = np.empty((C, C), np.float64)
    for kb in range(NB):
        cols = slice(kb * BS, (kb + 1) * BS)
        W1r[:, cols] = Fr[:, cols] @ w1[0, kb] - Fi[:, cols] @ w1[1, kb]
        W1i[:, cols] = Fi[:, cols] @ w1[0, kb] + Fr[:, cols] @ w1[1, kb]
    # IFFT (real part): out = o2r @ Gr + o2i @ Gi, G[k, c]
    Gr = Fr.T.copy()           # cos(2pi k c / C)/sqrt(C)
    Gi = Fi.T.copy()           # -sin(2pi k c / C)/sqrt(C)

    b1 = np.asarray(b1, np.float64)
    b2 = np.asarray(b2, np.float64)
    w2 = np.asarray(w2, np.float64)
    bf = ml_dtypes.bfloat16
    f8 = ml_dtypes.float8_e4m3

    if fp8:
        s1 = SX * SW1
        f8w = f8
    else:
        s1 = 1.0
        SW1_, SO_, SG_ = 1.0, 1.0, 1.0
        f8w = bf
    sw1 = SW1 if fp8 else 1.0
    so = SO if fp8 else 1.0
    sg = SG if fp8 else 1.0

    def swi_pair(w0, w1):
        # DoubleRowSwInterleave stream layout: per 128-col pair (W0, W1) the
        # 256-elem weight stream must read W0[127],W1[127],W0[126],...,W1[0].
        # The AP walks piece0 then piece1, so piece0 = first 128 stream elems.
        st = np.empty(w0.shape[:-1] + (256,), np.float64)
        st[..., 0::2] = w0[..., ::-1]
        st[..., 1::2] = w1[..., ::-1]
        return st[..., :128], st[..., 128:]

    def pack_pairs(a, b):
        # [NB, BS, X] x2 -> [NB, BS, 2, X]
        return np.ascontiguousarray(np.stack([a, b], axis=2))

    C2 = HIDDEN // 2
    # half-spectrum: A = o2r @ cos[k, c<512], B = o2i @ sin[k, c<512]
    # gsp holds -sin so that K3*pB = -B
    Ghc = (Gr[:, :C2] * sg).reshape(NB, BS, C2)
    Ghs = (Gi[:, :C2] * sg).reshape(NB, BS, C2)       # Gi = -sin -> pB = -SO*SG*B
    gcp = np.stack([Ghc[0::2], Ghc[1::2]], axis=2)    # [4, BS, 2, C2]
    gsp = np.stack([Ghs[0::2], Ghs[1::2]], axis=2)
    alt = (-1.0) ** k
    W1rs = (W1r * sw1).reshape(NB, BS, HIDDEN)
    W1is = (W1i * sw1).reshape(NB, BS, HIDDEN)
    C2 = HIDDEN // 2
    Ghc = (Gr[:, :C2] * sg).reshape(NB, BS, C2)
    Ghs = (Gi[:, :C2] * sg).reshape(NB, BS, C2)       # Gi = -sin -> pB = -SO*SG*B
    w2r0, w2r1 = w2[0] * so, -w2[1] * so              # [NB, BS, BS] each
    w2i0, w2i1 = w2[1] * so, w2[0] * so
    alt = (-1.0) ** k

    if SWI and fp8 and False:  # HW does the interleave itself (interp-layout feed failed)
        # per-matmul lhsT pair subtiles get the interleaved-reversed layout
        def swi_blocked(wa, wb):
            # wa/wb: [G, BS, X] pair halves; interleave per 128-col subtile
            G, P, X = wa.shape
            a = wa.reshape(G, P, X // BS, BS)
            b = wb.reshape(G, P, X // BS, BS)
            p0, p1 = swi_pair(a, b)
            return p0.reshape(G, P, X), p1.reshape(G, P, X)

        # w1: pairs are (ci=2t, ci=2t+1) blocks, M-subtiles are the ro cols
        w1r_pairs = swi_blocked(W1rs[0::2], W1rs[1::2])
        W1rs = np.empty_like(W1rs)
        W1rs[0::2], W1rs[1::2] = w1r_pairs
        w1i_pairs = swi_blocked(W1is[0::2], W1is[1::2])
        W1is = np.empty_like(W1is)
        W1is[0::2], W1is[1::2] = w1i_pairs
        # gcp/gsp: pairs are (kb=2t, kb=2t+1), M-subtiles are the co cols
        gc0, gc1 = swi_blocked(Ghc[0::2], Ghc[1::2])
        gs0, gs1 = swi_blocked(Ghs[0::2], Ghs[1::2])
        gcp = np.stack([gc0, gc1], axis=2)
        gsp = np.stack([gs0, gs1], axis=2)
        # w2: pairs are the (r,i) halves, single 128-col subtile
        w2r0, w2r1 = swi_pair(w2r0, w2r1)
        w2i0, w2i1 = swi_pair(w2i0, w2i1)
    else:
        gcp = np.stack([Ghc[0::2], Ghc[1::2]], axis=2)
        gsp = np.stack([Ghs[0::2], Ghs[1::2]], axis=2)

    out = {
        "w1r": np.ascontiguousarray(W1rs).astype(f8w),
        "w1i": np.ascontiguousarray(W1is).astype(f8w),
        "gcp": np.ascontiguousarray(gcp).astype(f8w),
        "gsp": np.ascontiguousarray(gsp).astype(f8w),
        "g512": np.ascontiguousarray(alt.reshape(NB, BS).T).astype(f8w),
        "w2rp": pack_pairs(w2r0, w2r1).astype(f8w),
        "w2ip": pack_pairs(w2i0, w2i1).astype(f8w),
        "b1r": np.ascontiguousarray(b1[0].T).astype(np.float32),
        "b1i": np.ascontiguousarray(b1[1].T).astype(np.float32),
        "b2m1r": np.ascontiguousarray((so * (b2[0] - LAM)).T).astype(np.float32),
        "b2m1i": np.ascontiguousarray((so * (b2[1] - LAM)).T).astype(np.float32),
        "b2m2r": np.ascontiguousarray((so * (-b2[0] - LAM)).T).astype(np.float32),
        "b2m2i": np.ascontiguousarray((so * (-b2[1] - LAM)).T).astype(np.float32),
    }
    return out


def _make_in_maps(x, w1, b1, w2, b2):
    x = np.asarray(x, np.float32)
    B = x.shape[0]
    weights = _build_host_weights(w1, b1, w2, b2)
    in_maps = []
    qdt = ml_dtypes.float8_e4m3 if FP8 else ml_dtypes.bfloat16
    qs = SX if FP8 else 1.0
    rev_idx = (HIDDEN - np.arange(HIDDEN // 2)) % HIDDEN   # c -> 1024-c (c=0 unused)
    for b in range(B):
        m = dict(weights)
        xt_b = np.ascontiguousarray(x[b].T)
        m["xt"] = xt_b.reshape(NB, BS, NROWS)
        m["xt8"] = (xt_b * qs).astype(qdt).reshape(NB, BS, NROWS)
        m["xtr"] = np.ascontiguousarray(xt_b[rev_idx]).reshape(NB // 2, BS, NROWS)
        in_maps.append(m)
    return in_maps


def _run(x, w1, b1, w2, b2, trace=False):
    nc = _build_graph()
    x = np.asarray(x, np.float32)
    B = x.shape[0]
    in_maps = _make_in_maps(x, w1, b1, w2, b2)
    res = run_bass_kernel_spmd(nc, in_maps, core_ids=list(range(N_CORES)),
                               trace=trace)
    outs = np.empty_like(x)
    out_t = np.empty((HIDDEN, NROWS), np.float32)
    for b in range(B):
        r = res.results[b]
        out_t[:HIDDEN // 2] = r["outl"].reshape(HIDDEN // 2, NROWS)
        out_t[HIDDEN // 2] = r["out5"][0]
        hi = r["outh"].reshape(HIDDEN // 2, NROWS)
        out_t[HIDDEN // 2 + 1:] = hi[1:][::-1]
        outs[b] = out_t.T
    return outs, res


def kernel(x, w1, b1, w2, b2):
    outs, _ = _run(x, w1, b1, w2, b2, trace=False)
    return outs
